# revision 10
# baseline (speedup 1.0000x reference)
"""DeepseekV2 MLA attention (B=1, S=2048, H=4096, 32 heads) on 8 Trainium2
NeuronCores.

Sharding: tensor-parallel over heads (4 heads/core) for q_b/kv_b/o_w; the
small LoRA-A projections are data-parallel over sequence, with on-device
AllGathers. o-proj partials (row-parallel) are summed on the host.

Schedule notes:
- q_lora ships RAW (unnormalized) in two half-rank AllGathers issued as
  early as possible (the first collective pays a ~50-80us bootstrap
  barrier); the rmsnorm 1/sqrt factor rides along as an extra row and is
  applied on the consumer at the q_b PSUM->SBUF copy. LayerNorm weights
  are folded into the B matrices host-side (exact).
- q_b contracts rank 0-767 as soon as the first half lands, dovetailing
  with the second half's gather.
- Attention processes head PAIRS: the two K=64 rope matmuls of a pair are
  row-tiled into disjoint halves of the PE array and run concurrently;
  exp is one 1024-wide ACT across both heads' PSUM banks; the causal
  diagonal narrows matmul/exp/accumulate widths to the unmasked range.
- o-proj tiles are emitted interleaved with attention so PE bubbles from
  the softmax dependency chain are filled; o_w stays SBUF-resident.
- All matmuls fp16 with fp32 PSUM; statistics fp32; outputs fp16.
"""
import contextlib
from collections import deque
import numpy as np

import concourse.bass as bass
import concourse.mybir as mybir
import concourse.tile as tile
from concourse import bacc
from concourse.bass_utils import run_bass_kernel_spmd

F32 = mybir.dt.float32
F16 = mybir.dt.float16
AF = mybir.ActivationFunctionType
OP = mybir.AluOpType

P = 128
H = 4096
NH = 32
DN, DR, DV = 128, 64, 128
QK = DN + DR            # 192
RQ, RKV = 1536, 512
EPS = 1e-6
NCORES = 8
NHL = NH // NCORES      # 4 heads per core
SCALE = QK ** -0.5
RQH = RQ // 2           # 768 rows per ag_q half

_BUILD_CACHE = {}


def build(S=2048):
    R = S // NCORES          # rows (seq positions) per core in phase 1
    SB = S // 512            # 4    512-wide seq blocks
    KB = S // 128            # 16   128-wide seq blocks
    HKB = H // 128           # 32   hidden contraction blocks
    QMB = (NHL * QK) // 128  # 6    q_b output row-blocks
    RQB = RQ // 128          # 12
    RKB = RKV // 128         # 4
    NOB = H // 512           # 8    o-proj output col-blocks

    nc = bacc.Bacc("TRN2", target_bir_lowering=False, num_devices=NCORES)

    hid_t = nc.declare_dram_parameter("hid_t", [P, HKB, R], F16, isOutput=False)
    cos4_in = nc.declare_dram_parameter("cos4", [P, S], F32, isOutput=False)
    sin4_in = nc.declare_dram_parameter("sin4", [P, S], F32, isOutput=False)
    cosl_in = nc.declare_dram_parameter("cosl", [DR, R], F32, isOutput=False)
    sinl_in = nc.declare_dram_parameter("sinl", [DR, R], F32, isOutput=False)
    qaw_t = nc.declare_dram_parameter("qaw_t", [RQB, P, HKB, P], F16, isOutput=False)
    kvaw_t = nc.declare_dram_parameter("kvaw_t", [RKB, P, HKB, P], F16, isOutput=False)
    kvaw_rot = nc.declare_dram_parameter("kvaw_rot", [P, HKB, DR], F16, isOutput=False)
    qbw_t = nc.declare_dram_parameter("qbw_t", [RQB, P, NHL * QK], F16, isOutput=False)
    kvbw_t = nc.declare_dram_parameter("kvbw_t", [RKB, P, NHL * (DN + DV)], F16, isOutput=False)
    ow_t = nc.declare_dram_parameter("ow_t", [NOB, P, NHL, 512], F16, isOutput=False)
    mask_in = nc.declare_dram_parameter("mask", [P, 896], F16, isOutput=False)

    o_part = nc.declare_dram_parameter("o_part", [S, H], F16, isOutput=True)

    # allgather buffers (fp16). q ships raw in four 3-block pieces so the
    # first transfer starts ~25us in and q_b dovetails with later pieces;
    # the last piece carries the per-position 1/rms row at index 384.
    ag_q_in = [nc.dram_tensor(f"ag_q_in{i}", [768 + (i == 1), R], F16)
               for i in range(2)]
    ag_q_out = [nc.dram_tensor(f"ag_q_out{i}", [NCORES, 768 + (i == 1), R],
                               F16, addr_space="Shared") for i in range(2)]
    ag_ckv_in = nc.dram_tensor("ag_ckv_in", [RKV + DR, R], F16)
    ag_ckv_out = nc.dram_tensor("ag_ckv_out", [NCORES, RKV + DR, R], F16,
                                addr_space="Shared")
    GROUPS = [list(range(NCORES))]

    with tile.TileContext(nc) as tc:
        _emit(nc, tc, locals())
    nc.compile()
    return nc


def _emit(nc, tc, ns):
    S = ns["S"]; R = ns["R"]; SB = ns["SB"]; KB = ns["KB"]; HKB = ns["HKB"]
    QMB = ns["QMB"]; RQB = ns["RQB"]; RKB = ns["RKB"]; NOB = ns["NOB"]
    hid_t = ns["hid_t"]
    cos4_in = ns["cos4_in"]; sin4_in = ns["sin4_in"]
    cosl_in = ns["cosl_in"]; sinl_in = ns["sinl_in"]
    qaw_t = ns["qaw_t"]; kvaw_t = ns["kvaw_t"]; kvaw_rot = ns["kvaw_rot"]
    qbw_t = ns["qbw_t"]; kvbw_t = ns["kvbw_t"]; ow_t = ns["ow_t"]
    mask_in = ns["mask_in"]; o_part = ns["o_part"]
    ag_q_in = ns["ag_q_in"]; ag_q_out = ns["ag_q_out"]
    ag_ckv_in = ns["ag_ckv_in"]; ag_ckv_out = ns["ag_ckv_out"]
    GROUPS = ns["GROUPS"]

    def ag(name, src, dst):
        with nc.named_scope(name):
            nc.gpsimd.collective_compute(
                "AllGather", OP.bypass, replica_groups=GROUPS,
                ins=[src[:]], outs=[dst[:]])

    ctx = contextlib.ExitStack()
    with ctx:
        const = ctx.enter_context(tc.tile_pool(name="const", bufs=1))

        wres = ctx.enter_context(tc.tile_pool(name="wres", bufs=1))

        hidp_ctx = contextlib.ExitStack()
        hidp = hidp_ctx.enter_context(tc.tile_pool(name="hidp", bufs=1))
        hid_c = [hidp.tile([P, 8, R], F16, tag=f"hid{c}", name=f"hid{c}")
                 for c in range(4)]
        nc.sync.dma_start(hid_c[0][:], hid_t[:, 0:8, :])

        def hid_all(kb):
            return hid_c[kb // 8][:, kb % 8, :]

        ones_f = const.tile([P, 1], F32, tag="onesf")
        nc.vector.memset(ones_f[:], 1.0)
        ones_col = const.tile([P, 1], F16, tag="ones")
        nc.vector.tensor_copy(ones_col[:], ones_f[:])

        # ============ phase 1: LoRA-A projections (this core's R rows) ======
        p1ctx = contextlib.ExitStack()
        p1 = p1ctx.enter_context(tc.tile_pool(name="p1", bufs=2))
        p1sq = p1ctx.enter_context(tc.tile_pool(name="p1sq", bufs=4))
        p1w = p1ctx.enter_context(tc.tile_pool(name="p1w", bufs=3))
        p1ps = p1ctx.enter_context(tc.tile_pool(name="p1ps", bufs=2, space="PSUM"))
        p1ss = p1ctx.enter_context(tc.tile_pool(name="p1ss", bufs=2, space="PSUM"))

        qss_ps = p1ss.tile([1, R], F32, tag="qss")

        def qa_piece(pc):
            """q_a rows [pc*768, (pc+1)*768): matmul, ship RAW, accum sq."""
            with nc.named_scope(f"ph1_qa{pc}"):
                for i in range(6):
                    mb = 6 * pc + i
                    if mb == 0:
                        for c in range(1, 4):
                            nc.sync.dma_start(hid_c[c][:],
                                              hid_t[:, 8 * c:8 * (c + 1), :])
                    ps = p1ps.tile([P, R], F32, tag="p1ps")
                    w = p1w.tile([P, HKB, P], F16, tag="w")
                    nc.sync.dma_start(w[:], qaw_t[mb])
                    for kb in range(HKB):
                        nc.tensor.matmul(ps[:], w[:, kb, :], hid_all(kb),
                                         start=(kb == 0), stop=(kb == HKB - 1))
                    raw = p1.tile([P, R], F16, tag="raw")
                    nc.vector.tensor_copy(raw[:], ps[:])
                    sq = p1sq.tile([P, R], F16, tag="sq")
                    nc.vector.tensor_tensor(sq[:], raw[:], raw[:], OP.mult)
                    nc.tensor.matmul(qss_ps[:], ones_col[:], sq[:],
                                     start=(mb == 0), stop=(mb == RQB - 1))
                    nc.sync.dma_start(ag_q_in[pc][i * P:(i + 1) * P, :], raw[:])

        for pc in range(2):
            qa_piece(pc)
            if pc == 0:
                # prefetch the later-phase weights behind the q_a stream
                qbw = wres.tile([P, RQB, NHL * QK], F16, tag="qbw")
                for kb in range(RQB):
                    nc.sync.dma_start(qbw[:, kb, :], qbw_t[kb])
                kvbw = wres.tile([P, RKB, NHL * (DN + DV)], F16, tag="kvbw")
                for b in range(RKB):
                    nc.sync.dma_start(kvbw[:, b, :], kvbw_t[b])
                mask_sb = const.tile([P, 896], F16, tag="mask")
                nc.sync.dma_start(mask_sb[:], mask_in[:])
            if pc == 1:
                # q 1/rms row -> tail of the last piece
                qinv = p1.tile([1, R], F16, tag="qinv")
                qi32 = p1.tile([1, R], F32, tag="qi32")
                nc.vector.tensor_scalar(qi32[:], qss_ps[:], 1.0 / RQ, EPS,
                                        OP.mult, OP.add)
                nc.scalar.activation(qi32[:], qi32[:], AF.Sqrt)
                nc.vector.reciprocal(qi32[:], qi32[:])
                nc.vector.tensor_copy(qinv[:], qi32[:])
                nc.sync.dma_start(ag_q_in[1][768:769, :], qinv[:])
            ag(f"ag_q{pc}", ag_q_in[pc], ag_q_out[pc])

        # ---- kv lora (normalized locally; ln folded into kv_b host-side) ---
        with nc.named_scope("ph1_kva"):
            kss_ps = p1ss.tile([1, R], F32, tag="kss")
            cp_all = p1.tile([P, RKB, R], F32, tag="cpkv")
            for mb in range(RKB):
                ps = p1ps.tile([P, R], F32, tag="p1ps")
                w = p1w.tile([P, HKB, P], F16, tag="w")
                nc.sync.dma_start(w[:], kvaw_t[mb])
                for kb in range(HKB):
                    nc.tensor.matmul(ps[:], w[:, kb, :], hid_all(kb),
                                     start=(kb == 0), stop=(kb == HKB - 1))
                nc.scalar.copy(cp_all[:, mb, :], ps[:])
                sq = p1sq.tile([P, R], F16, tag="sq")
                nc.vector.tensor_tensor(sq[:], cp_all[:, mb, :],
                                        cp_all[:, mb, :], OP.mult)
                nc.tensor.matmul(kss_ps[:], ones_col[:], sq[:],
                                 start=(mb == 0), stop=(mb == RKB - 1))
            # shared rope key
            cosl = p1.tile([DR, R], F32, tag="cosl")
            nc.sync.dma_start(cosl[:], cosl_in[:])
            sinl = p1.tile([DR, R], F32, tag="sinl")
            nc.sync.dma_start(sinl[:], sinl_in[:])
            rps = p1ps.tile([DR, R], F32, tag="rps")
            wr = p1w.tile([P, HKB, DR], F16, tag="wr")
            nc.sync.dma_start(wr[:], kvaw_rot[:])
            for kb in range(HKB):
                nc.tensor.matmul(rps[:], wr[:, kb, :], hid_all(kb),
                                 start=(kb == 0), stop=(kb == HKB - 1))
            kr = p1.tile([DR, R], F32, tag="kr")
            nc.vector.tensor_copy(kr[:], rps[:])
            pa = p1.tile([DR, R], F32, tag="pa")
            nc.vector.tensor_tensor(pa[0:32], kr[0:32], cosl[0:32], OP.mult)
            nc.vector.tensor_tensor(pa[32:64], kr[32:64], sinl[32:64], OP.mult)
            pb = p1.tile([DR, R], F32, tag="pb")
            nc.vector.tensor_tensor(pb[0:32], kr[0:32], sinl[0:32], OP.mult)
            nc.vector.tensor_tensor(pb[32:64], kr[32:64], cosl[32:64], OP.mult)
            sh_a = p1.tile([32, R], F32, tag="sha")
            nc.sync.dma_start(sh_a[:], pa[32:64])
            sh_b = p1.tile([32, R], F32, tag="shb")
            nc.sync.dma_start(sh_b[:], pb[32:64])
            out_r = p1.tile([32, R], F16, tag="outr")
            nc.vector.tensor_sub(out_r[:], pa[0:32], sh_a[:])
            out_i = p1.tile([32, R], F16, tag="outi")
            nc.vector.tensor_add(out_i[:], pb[0:32], sh_b[:])
            nc.sync.dma_start(ag_ckv_in[RKV:RKV + 32, :], out_r[:])
            nc.sync.dma_start(ag_ckv_in[RKV + 32:RKV + DR, :], out_i[:])
            # kv rmsnorm (local)
            kinv = p1.tile([1, R], F32, tag="kinv")
            nc.vector.tensor_scalar(kinv[:], kss_ps[:], 1.0 / RKV, EPS,
                                    OP.mult, OP.add)
            nc.scalar.activation(kinv[:], kinv[:], AF.Sqrt)
            nc.vector.reciprocal(kinv[:], kinv[:])
            kinv_bc = p1.tile([P, R], F32, tag="kinvbc")
            nc.gpsimd.partition_broadcast(kinv_bc[:], kinv[:])
            for mb in range(RKB):
                outn = p1.tile([P, R], F16, tag="outn")
                nc.vector.tensor_tensor(outn[:], cp_all[:, mb, :], kinv_bc[:],
                                        OP.mult)
                nc.sync.dma_start(ag_ckv_in[mb * P:(mb + 1) * P, :], outn[:])
        ag("ag_ckv", ag_ckv_in, ag_ckv_out)
        p1ctx.close()
        hidp_ctx.close()

        # rope tables / mask (host-precomputed), prefetched during phase 1
        cos4 = const.tile([P, S], F32, tag="cos4")
        nc.sync.dma_start(cos4[:], cos4_in[:])
        sin4 = const.tile([P, S], F32, tag="sin4")
        nc.sync.dma_start(sin4[:], sin4_in[:])

        # ============ phase 2b: q_b -> q_passT + roped q_rot ================
        qres = ctx.enter_context(tc.tile_pool(name="qres", bufs=1))
        qpass = [qres.tile([P, S], F16, tag=f"qp{h}", name=f"qp{h}")
                 for h in range(NHL)]
        q_re = qres.tile([P, S], F16, tag="qre")   # [re_h0..re_h3] x32, roped
        q_im = qres.tile([P, S], F16, tag="qim")   # [im_h0..im_h3] x32, roped

        with (
            tc.tile_pool(name="qlp", bufs=1) as qlp,
            tc.tile_pool(name="qstg", bufs=2) as qstg,
            tc.tile_pool(name="ropep", bufs=2) as ropep,
            tc.tile_pool(name="qbps", bufs=2, space="PSUM") as qbps,
            nc.named_scope("ph2_qb"),
        ):
            # gather-side loads, emitted in collective completion order
            ql_pc = [qlp.tile([P, 6, S], F16, tag=f"ql{pc}", name=f"ql{pc}")
                     for pc in range(2)]
            for pc in range(2):
                for j in range(6):
                    nc.sync.dma_start(
                        ql_pc[pc][:, j, :].rearrange("p (r c) -> p r c",
                                                     r=NCORES),
                        ag_q_out[pc][:, j * P:(j + 1) * P, :]
                        .rearrange("r p c -> p r c"))
            qiv = qlp.tile([1, S], F16, tag="qiv")
            nc.sync.dma_start(
                qiv[:].rearrange("p (r c) -> p r c", r=NCORES),
                ag_q_out[1][:, 768:769, :].rearrange("r p c -> p r c"))
            qiv32 = qlp.tile([1, S], F32, tag="qiv32")
            nc.vector.tensor_copy(qiv32[:], qiv[:])
            qinv_bc = qres.tile([P, S], F32, tag="qinvbc")
            nc.gpsimd.partition_broadcast(qinv_bc[:], qiv32[:])

            HQ = QMB // 2
            for sb in range(SB):
                sl = slice(sb * 512, (sb + 1) * 512)
                qre32 = qstg.tile([P, 512], F32, tag="qre32")
                qim32 = qstg.tile([P, 512], F32, tag="qim32")
                for rep in range(2):
                    pss = [qbps.tile([P, 512], F32, tag=f"qps{j}",
                                     name=f"qps{j}") for j in range(HQ)]
                    for kb in range(RQB):
                        ql = ql_pc[kb // 6]
                        for j in range(HQ):
                            m = rep * HQ + j
                            nc.tensor.matmul(
                                pss[j][:], qbw[:, kb, m * P:(m + 1) * P],
                                ql[:, kb % 6, sl],
                                start=(kb == 0), stop=(kb == RQB - 1))
                    for j in range(HQ):
                        m = rep * HQ + j
                        if m < NHL:
                            dst = qpass[m][:, sl]
                        elif m == NHL:
                            dst = qre32[:]
                        else:
                            dst = qim32[:]
                        nc.vector.tensor_tensor(dst, pss[j][:],
                                                qinv_bc[:, sl], OP.mult)
                # rope this 512-chunk (fp32 math, fp16 results)
                t = ropep.tile([P, 512], F32, tag="t")
                u = ropep.tile([P, 512], F32, tag="u")
                t2 = ropep.tile([P, 512], F32, tag="t2")
                nc.vector.tensor_tensor(t[:], qre32[:], cos4[:, sl], OP.mult)
                nc.vector.tensor_tensor(u[:], qim32[:], sin4[:, sl], OP.mult)
                nc.vector.tensor_tensor(t2[:], qre32[:], sin4[:, sl], OP.mult)
                nc.vector.tensor_sub(q_re[:, sl], t[:], u[:])
                nc.vector.tensor_tensor(u[:], qim32[:], cos4[:, sl], OP.mult)
                nc.vector.tensor_add(q_im[:, sl], t2[:], u[:])

        # ============ phase 2a: kv_b -> k_passT, v ==========================
        kvres = ctx.enter_context(tc.tile_pool(name="kvres", bufs=1))
        kpass = [kvres.tile([P, S], F16, tag=f"kp{h}", name=f"kp{h}")
                 for h in range(NHL)]
        krot2 = kvres.tile([P, S], F16, tag="krot2")   # krot duplicated 2x64
        v_all = kvres.tile([P, KB, 512], F16, tag="v")
        ow_sb = kvres.tile([P, NOB, NHL, 512], F16, tag="owsb")

        with (
            tc.tile_pool(name="ckvp", bufs=1) as ckvp,
            tc.tile_pool(name="p2ps", bufs=3, space="PSUM") as p2ps,
            nc.named_scope("ph2_kvb"),
        ):
            ckv_sb = ckvp.tile([P, RKB, S], F16, tag="ckv")
            for b in range(RKB):
                nc.sync.dma_start(
                    ckv_sb[:, b, :].rearrange("p (r c) -> p r c", r=NCORES),
                    ag_ckv_out[:, b * P:(b + 1) * P, :].rearrange("r p c -> p r c"))
            for half in range(2):
                nc.sync.dma_start(
                    krot2[64 * half:64 * (half + 1), :]
                    .rearrange("p (r c) -> p r c", r=NCORES),
                    ag_ckv_out[:, RKV:RKV + DR, :].rearrange("r p c -> p r c"))
            for nb in range(NOB):
                nc.sync.dma_start(ow_sb[:, nb, :, :], ow_t[nb])

            for h in range(NHL):
                for sb in range(SB):
                    ps = p2ps.tile([P, 512], F32, tag="ps2")
                    for b in range(RKB):
                        nc.tensor.matmul(
                            ps[:], kvbw[:, b, h * P:(h + 1) * P],
                            ckv_sb[:, b, sb * 512:(sb + 1) * 512],
                            start=(b == 0), stop=(b == RKB - 1))
                    nc.vector.tensor_copy(kpass[h][:, sb * 512:(sb + 1) * 512],
                                          ps[:])
            for s in range(KB):
                ps = p2ps.tile([P, 512], F32, tag="ps2")
                for b in range(RKB):
                    nc.tensor.matmul(
                        ps[:], ckv_sb[:, b, s * P:(s + 1) * P],
                        kvbw[:, b, NHL * DN:],
                        start=(b == 0), stop=(b == RKB - 1))
                nc.vector.tensor_copy(v_all[:, s, :], ps[:])

        # ============ phase 3+4: attention with interleaved o-proj ==========
        ores = ctx.enter_context(tc.tile_pool(name="ores", bufs=1))
        o_heads = [[ores.tile([P, 512], F16, tag=f"oh{h}_{qb}",
                              name=f"oh{h}_{qb}") for qb in range(SB)]
                   for h in range(NHL)]

        jobs = deque()

        with (
            tc.tile_pool(name="scps", bufs=2, space="PSUM") as scps,
            tc.tile_pool(name="ops", bufs=1, space="PSUM") as ops,
            tc.tile_pool(name="smps", bufs=1, space="PSUM") as smps,
            tc.tile_pool(name="ojps", bufs=1, space="PSUM") as ojps,
            tc.tile_pool(name="att", bufs=3) as att,
            tc.tile_pool(name="atts", bufs=4) as atts,
            tc.tile_pool(name="attq", bufs=2) as attq,
            tc.tile_pool(name="oj", bufs=3) as oj,
            nc.named_scope("ph34"),
        ):
            def emit_job():
                qb, s, nb = jobs.popleft()
                ps = ojps.tile([P, 512], F32, tag="ojps")
                for h in range(NHL):
                    nc.tensor.matmul(
                        ps[:], o_heads[h][qb][:, s * P:(s + 1) * P],
                        ow_sb[:, nb, h, :],
                        start=(h == 0), stop=(h == NHL - 1))
                ot = oj.tile([P, 512], F16, tag="ot")
                nc.vector.tensor_copy(ot[:], ps[:])
                nc.sync.dma_start(
                    o_part[qb * 512 + s * P: qb * 512 + (s + 1) * P,
                           nb * 512:(nb + 1) * 512], ot[:])

            def drain(k):
                for _ in range(min(k, len(jobs))):
                    emit_job()

            for qb in range(SB):
                nkb = (qb + 1) * 4
                for hp in range(NHL // 2):
                    h0, h1 = 2 * hp, 2 * hp + 1
                    # packed roped-q for the pair: [re0;im0;re1;im1] x32
                    qrs = attq.tile([P, 512], F16, tag="qrs")
                    for j, h in ((0, h0), (2, h1)):
                        nc.sync.dma_start(
                            qrs[32 * j:32 * (j + 1), :],
                            q_re[32 * h:32 * (h + 1), qb * 512:(qb + 1) * 512])
                        nc.sync.dma_start(
                            qrs[32 * (j + 1):32 * (j + 2), :],
                            q_im[32 * h:32 * (h + 1), qb * 512:(qb + 1) * 512])
                    o_ps = [ops.tile([P, 512], F32, tag=f"ops{j}",
                                     name=f"ops{j}") for j in range(2)]
                    spart = [atts.tile([P, 512], F16, tag=f"sp{j}",
                                       name=f"sp{j}") for j in range(2)]
                    for kb in range(nkb):
                        k_sl = slice(kb * P, (kb + 1) * P)
                        d = kb * P - qb * 512
                        w0 = max(d, 0)
                        psl = slice(w0, 512)
                        qsl = slice(qb * 512 + w0, (qb + 1) * 512)
                        s_ps = scps.tile([P, 2, 512], F32, tag="sps")
                        for j, h in ((0, h0), (1, h1)):
                            nc.tensor.matmul(s_ps[:, j, psl],
                                             kpass[h][:, k_sl],
                                             qpass[h][:, qsl],
                                             start=True, stop=False)
                        for j in range(2):
                            nc.tensor.matmul(s_ps[:, j, psl],
                                             krot2[64 * j:64 * (j + 1), k_sl],
                                             qrs[64 * j:64 * (j + 1), psl],
                                             start=False, stop=True)
                        probs = att.tile([P, 2, 512], F16, tag="probs")
                        nc.scalar.activation(probs[:, :, psl], s_ps[:, :, psl],
                                             AF.Exp, scale=SCALE)
                        for j in range(2):
                            if d >= 0:
                                nc.vector.tensor_tensor(
                                    probs[:, j, w0:w0 + P],
                                    probs[:, j, w0:w0 + P],
                                    mask_sb[:, 384:512], OP.mult)
                            if kb == 0:
                                nc.vector.tensor_copy(spart[j][:],
                                                      probs[:, j, :])
                            else:
                                nc.vector.tensor_tensor(spart[j][:, psl],
                                                        spart[j][:, psl],
                                                        probs[:, j, psl],
                                                        OP.add)
                            nc.tensor.matmul(o_ps[j][:, psl],
                                             v_all[:, kb, (h0 + j) * P:
                                                   (h0 + j + 1) * P],
                                             probs[:, j, psl],
                                             start=(kb == 0),
                                             stop=(kb == nkb - 1))
                        drain(1)
                    for j in range(2):
                        sm_ps = smps.tile([1, 512], F32, tag="smps")
                        nc.tensor.matmul(sm_ps[:], ones_col[:], spart[j][:],
                                         start=True, stop=True)
                        rec = att.tile([1, 512], F32, tag="rec")
                        nc.vector.reciprocal_approx_fast(rec[:], sm_ps[:])
                        rec_bc = att.tile([P, 512], F32, tag="recbc")
                        nc.gpsimd.partition_broadcast(rec_bc[:], rec[:])
                        nc.vector.tensor_tensor(o_heads[h0 + j][qb][:],
                                                o_ps[j][:], rec_bc[:],
                                                OP.mult)
                    drain(2)
                for s in range(4):
                    for nb in range(NOB):
                        jobs.append((qb, s, nb))
            drain(len(jobs))


# ======================= host-side prep & entry ==========================

def prep_inputs(hidden_states, freqs, q_a_w, q_a_ln_w, q_b_w, kv_a_w,
                kv_a_ln_w, kv_b_w, o_w):
    S = hidden_states.shape[1]
    R = S // NCORES
    f32, f16 = np.float32, np.float16

    hidT = np.ascontiguousarray(hidden_states[0].T.astype(f32))      # [H, S]
    freqsT = np.ascontiguousarray(freqs[0].T.astype(f32))            # [32, S]
    cos4 = np.ascontiguousarray(np.tile(np.cos(freqsT), (4, 1)))
    sin4 = np.ascontiguousarray(np.tile(np.sin(freqsT), (4, 1)))

    qawT = q_a_w.astype(f32).T                                       # [H, RQ]
    qaw_t = np.ascontiguousarray(
        qawT.reshape(H // P, P, RQ // P, P).transpose(2, 1, 0, 3).astype(f16))

    kva = kv_a_w.astype(f32)
    kva_main = kva[:RKV]
    kva_rot = np.concatenate([kva[RKV:][0::2], kva[RKV:][1::2]], axis=0)
    kvaw_t = np.ascontiguousarray(
        kva_main.T.reshape(H // P, P, RKV // P, P).transpose(2, 1, 0, 3).astype(f16))
    kvaw_rot_t = np.ascontiguousarray(
        kva_rot.T.reshape(H // P, P, DR).transpose(1, 0, 2).astype(f16))

    # big causal mask [128, 896]: M[p, g] = 1 if g >= p + 384
    g = np.arange(896)[None, :]
    p = np.arange(P)[:, None]
    mask = (g >= p + 384).astype(f16)

    # fold layernorm weights into the B matrices (exact: rmsnorm's 1/rms is
    # applied separately; the elementwise ln scale commutes into B columns)
    qbw = q_b_w.astype(f32) * q_a_ln_w.astype(f32)[None, :]
    kvbw = kv_b_w.astype(f32) * kv_a_ln_w.astype(f32)[None, :]
    ow = o_w.astype(f32)

    in_maps = []
    for c in range(NCORES):
        heads = list(range(NHL * c, NHL * (c + 1)))
        qb_pass = np.concatenate([qbw[QK * h:QK * h + DN] for h in heads], axis=0)
        qb_re = np.concatenate(
            [qbw[QK * h + DN:QK * (h + 1)][0::2] for h in heads], axis=0)
        qb_im = np.concatenate(
            [qbw[QK * h + DN:QK * (h + 1)][1::2] for h in heads], axis=0)
        qb_core = np.concatenate([qb_pass, qb_re, qb_im], axis=0)    # [768, RQ]
        qbw_tc = np.ascontiguousarray(
            qb_core.T.reshape(RQ // P, P, NHL * QK).astype(f16))
        kp = np.concatenate(
            [kvbw[(DN + DV) * h:(DN + DV) * h + DN] for h in heads], axis=0)
        vv = np.concatenate(
            [kvbw[(DN + DV) * h + DN:(DN + DV) * (h + 1)] for h in heads], axis=0)
        kvb_core = np.concatenate([kp, vv], axis=0)                  # [1024, RKV]
        kvbw_tc = np.ascontiguousarray(
            kvb_core.T.reshape(RKV // P, P, NHL * (DN + DV)).astype(f16))
        ow_slice = ow[:, NHL * DV * c: NHL * DV * (c + 1)].T         # [512, H]
        ow_tc = np.ascontiguousarray(
            ow_slice.reshape(NHL, P, H // 512, 512).transpose(2, 1, 0, 3).astype(f16))

        hid_c = np.ascontiguousarray(
            hidT[:, R * c:R * (c + 1)].reshape(H // P, P, R)
            .transpose(1, 0, 2).astype(f16))
        fl = freqsT[:, R * c:R * (c + 1)]
        cosl_c = np.ascontiguousarray(np.tile(np.cos(fl), (2, 1)))
        sinl_c = np.ascontiguousarray(np.tile(np.sin(fl), (2, 1)))

        in_maps.append({
            "hid_t": hid_c,
            "cos4": cos4,
            "sin4": sin4,
            "cosl": cosl_c,
            "sinl": sinl_c,
            "qaw_t": qaw_t,
            "kvaw_t": kvaw_t,
            "kvaw_rot": kvaw_rot_t,
            "qbw_t": qbw_tc,
            "kvbw_t": kvbw_tc,
            "ow_t": ow_tc,
            "mask": mask,
        })
    return in_maps


def _run(inputs, trace=False, trace_kwargs=None):
    S = inputs["hidden_states"].shape[1]
    if S not in _BUILD_CACHE:
        _BUILD_CACHE[S] = build(S)
    nc = _BUILD_CACHE[S]
    in_maps = prep_inputs(**inputs)
    kw = {}
    if trace:
        kw["trace"] = True
        if trace_kwargs:
            kw.update(trace_kwargs)
    res = run_bass_kernel_spmd(nc, in_maps, list(range(NCORES)), **kw)
    parts = np.stack([r["o_part"] for r in res.results], axis=0)
    out = parts.astype(np.float64).sum(axis=0).astype(np.float32)
    return out[None], res


def kernel(**inputs):
    out, _ = _run(inputs)
    return out


# revision 13
# speedup vs baseline: 1.0559x; 1.0559x over previous
"""DeepseekV2 MLA attention (B=1, S=2048, H=4096, 32 heads) on 8 Trainium2
NeuronCores.

Sharding: tensor-parallel over heads (4 heads/core) for q_b/kv_b/o_w; the
small LoRA-A projections are data-parallel over sequence, with on-device
AllGathers. o-proj partials (row-parallel) are summed on the host.

Schedule notes:
- q_lora ships RAW (unnormalized) in two half-rank AllGathers issued as
  early as possible (the first collective pays a ~50-80us bootstrap
  barrier); the rmsnorm 1/sqrt factor rides along as an extra row and is
  applied on the consumer at the q_b PSUM->SBUF copy. LayerNorm weights
  are folded into the B matrices host-side (exact).
- q_b contracts rank 0-767 as soon as the first half lands, dovetailing
  with the second half's gather.
- Attention processes head PAIRS: the two K=64 rope matmuls of a pair are
  row-tiled into disjoint halves of the PE array and run concurrently;
  exp is one 1024-wide ACT across both heads' PSUM banks; the causal
  diagonal narrows matmul/exp/accumulate widths to the unmasked range.
- o-proj tiles are emitted interleaved with attention so PE bubbles from
  the softmax dependency chain are filled; o_w stays SBUF-resident.
- All matmuls fp16 with fp32 PSUM; statistics fp32; outputs fp16.
"""
import contextlib
from collections import deque
import numpy as np

import concourse.bass as bass
import concourse.mybir as mybir
import concourse.tile as tile
from concourse import bacc
from concourse.bass_utils import run_bass_kernel_spmd

F32 = mybir.dt.float32
F16 = mybir.dt.float16
AF = mybir.ActivationFunctionType
OP = mybir.AluOpType

P = 128
H = 4096
NH = 32
DN, DR, DV = 128, 64, 128
QK = DN + DR            # 192
RQ, RKV = 1536, 512
EPS = 1e-6
NCORES = 8
NHL = NH // NCORES      # 4 heads per core
SCALE = QK ** -0.5
RQH = RQ // 2           # 768 rows per ag_q half

_BUILD_CACHE = {}


def build(S=2048):
    R = S // NCORES          # rows (seq positions) per core in phase 1
    SB = S // 512            # 4    512-wide seq blocks
    KB = S // 128            # 16   128-wide seq blocks
    HKB = H // 128           # 32   hidden contraction blocks
    QMB = (NHL * QK) // 128  # 6    q_b output row-blocks
    RQB = RQ // 128          # 12
    RKB = RKV // 128         # 4
    NOB = H // 512           # 8    o-proj output col-blocks

    nc = bacc.Bacc("TRN2", target_bir_lowering=False, num_devices=NCORES)

    hid_t = nc.declare_dram_parameter("hid_t", [P, HKB, R], F16, isOutput=False)
    cos4_in = nc.declare_dram_parameter("cos4", [P, S], F32, isOutput=False)
    sin4_in = nc.declare_dram_parameter("sin4", [P, S], F32, isOutput=False)
    cosl_in = nc.declare_dram_parameter("cosl", [DR, R], F32, isOutput=False)
    sinl_in = nc.declare_dram_parameter("sinl", [DR, R], F32, isOutput=False)
    qaw_t = nc.declare_dram_parameter("qaw_t", [RQB, P, HKB, P], F16, isOutput=False)
    kvaw_t = nc.declare_dram_parameter("kvaw_t", [RKB, P, HKB, P], F16, isOutput=False)
    kvaw_rot = nc.declare_dram_parameter("kvaw_rot", [P, HKB, DR], F16, isOutput=False)
    qbw_t = nc.declare_dram_parameter("qbw_t", [RQB, P, NHL * QK], F16, isOutput=False)
    kvbw_t = nc.declare_dram_parameter("kvbw_t", [RKB, P, NHL * (DN + DV)], F16, isOutput=False)
    ow_t = nc.declare_dram_parameter("ow_t", [NOB, P, NHL, 512], F16, isOutput=False)
    mask_in = nc.declare_dram_parameter("mask", [P, 896], F16, isOutput=False)

    o_part = nc.declare_dram_parameter("o_part", [S, H], F16, isOutput=True)

    # allgather buffers (fp16). q ships raw in four 3-block pieces so the
    # first transfer starts ~25us in and q_b dovetails with later pieces;
    # the last piece carries the per-position 1/rms row at index 384.
    QP = [4, 8]              # q_a blocks per allgather piece
    ag_q_in = [nc.dram_tensor(f"ag_q_in{i}", [128 * QP[i] + (i == 1), R], F16)
               for i in range(2)]
    ag_q_out = [nc.dram_tensor(f"ag_q_out{i}",
                               [NCORES, 128 * QP[i] + (i == 1), R],
                               F16, addr_space="Shared") for i in range(2)]
    ag_ckv_in = nc.dram_tensor("ag_ckv_in", [RKV + DR, R], F16)
    ag_ckv_out = nc.dram_tensor("ag_ckv_out", [NCORES, RKV + DR, R], F16,
                                addr_space="Shared")
    GROUPS = [list(range(NCORES))]

    with tile.TileContext(nc) as tc:
        _emit(nc, tc, locals())
    nc.compile()
    return nc


def _emit(nc, tc, ns):
    S = ns["S"]; R = ns["R"]; SB = ns["SB"]; KB = ns["KB"]; HKB = ns["HKB"]
    QMB = ns["QMB"]; RQB = ns["RQB"]; RKB = ns["RKB"]; NOB = ns["NOB"]
    hid_t = ns["hid_t"]
    cos4_in = ns["cos4_in"]; sin4_in = ns["sin4_in"]
    cosl_in = ns["cosl_in"]; sinl_in = ns["sinl_in"]
    qaw_t = ns["qaw_t"]; kvaw_t = ns["kvaw_t"]; kvaw_rot = ns["kvaw_rot"]
    qbw_t = ns["qbw_t"]; kvbw_t = ns["kvbw_t"]; ow_t = ns["ow_t"]
    mask_in = ns["mask_in"]; o_part = ns["o_part"]
    ag_q_in = ns["ag_q_in"]; ag_q_out = ns["ag_q_out"]; QP = ns["QP"]
    ag_ckv_in = ns["ag_ckv_in"]; ag_ckv_out = ns["ag_ckv_out"]
    GROUPS = ns["GROUPS"]

    def ag(name, src, dst):
        with nc.named_scope(name):
            nc.gpsimd.collective_compute(
                "AllGather", OP.bypass, replica_groups=GROUPS,
                ins=[src[:]], outs=[dst[:]])

    ctx = contextlib.ExitStack()
    with ctx:
        const = ctx.enter_context(tc.tile_pool(name="const", bufs=1))

        wres = ctx.enter_context(tc.tile_pool(name="wres", bufs=1))

        hidp_ctx = contextlib.ExitStack()
        hidp = hidp_ctx.enter_context(tc.tile_pool(name="hidp", bufs=1))
        hid_c = [hidp.tile([P, 8, R], F16, tag=f"hid{c}", name=f"hid{c}")
                 for c in range(4)]
        nc.sync.dma_start(hid_c[0][:], hid_t[:, 0:8, :])

        def hid_all(kb):
            return hid_c[kb // 8][:, kb % 8, :]

        ones_f = const.tile([P, 1], F32, tag="onesf")
        nc.vector.memset(ones_f[:], 1.0)
        ones_col = const.tile([P, 1], F16, tag="ones")
        nc.vector.tensor_copy(ones_col[:], ones_f[:])

        # ============ phase 1: LoRA-A projections (this core's R rows) ======
        p1ctx = contextlib.ExitStack()
        p1 = p1ctx.enter_context(tc.tile_pool(name="p1", bufs=2))
        p1sq = p1ctx.enter_context(tc.tile_pool(name="p1sq", bufs=4))
        p1w = p1ctx.enter_context(tc.tile_pool(name="p1w", bufs=3))
        p1ps = p1ctx.enter_context(tc.tile_pool(name="p1ps", bufs=2, space="PSUM"))
        p1ss = p1ctx.enter_context(tc.tile_pool(name="p1ss", bufs=2, space="PSUM"))

        qss_ps = p1ss.tile([1, R], F32, tag="qss")

        # ---- kv lora first: its allgather takes the first CC slot so kv_b
        # can run while the (bigger) q gathers are still in flight ----
        with nc.named_scope("ph1_kva"):
            kss_ps = p1ss.tile([1, R], F32, tag="kss")
            cp_all = p1.tile([P, RKB, R], F32, tag="cpkv")
            for mb in range(RKB):
                ps = p1ps.tile([P, R], F32, tag="p1ps")
                w = p1w.tile([P, HKB, P], F16, tag="w")
                nc.sync.dma_start(w[:], kvaw_t[mb])
                if mb == 0:
                    for c in range(1, 4):
                        nc.sync.dma_start(hid_c[c][:],
                                          hid_t[:, 8 * c:8 * (c + 1), :])
                for kb in range(HKB):
                    nc.tensor.matmul(ps[:], w[:, kb, :], hid_all(kb),
                                     start=(kb == 0), stop=(kb == HKB - 1))
                nc.scalar.copy(cp_all[:, mb, :], ps[:])
                sq = p1sq.tile([P, R], F16, tag="sq")
                nc.vector.tensor_tensor(sq[:], cp_all[:, mb, :],
                                        cp_all[:, mb, :], OP.mult)
                nc.tensor.matmul(kss_ps[:], ones_col[:], sq[:],
                                 start=(mb == 0), stop=(mb == RKB - 1))
            # shared rope key
            cosl = p1.tile([DR, R], F32, tag="cosl")
            nc.sync.dma_start(cosl[:], cosl_in[:])
            sinl = p1.tile([DR, R], F32, tag="sinl")
            nc.sync.dma_start(sinl[:], sinl_in[:])
            rps = p1ps.tile([DR, R], F32, tag="rps")
            wr = p1w.tile([P, HKB, DR], F16, tag="wr")
            nc.sync.dma_start(wr[:], kvaw_rot[:])
            for kb in range(HKB):
                nc.tensor.matmul(rps[:], wr[:, kb, :], hid_all(kb),
                                 start=(kb == 0), stop=(kb == HKB - 1))
            kr = p1.tile([DR, R], F32, tag="kr")
            nc.vector.tensor_copy(kr[:], rps[:])
            pa = p1.tile([DR, R], F32, tag="pa")
            nc.vector.tensor_tensor(pa[0:32], kr[0:32], cosl[0:32], OP.mult)
            nc.vector.tensor_tensor(pa[32:64], kr[32:64], sinl[32:64], OP.mult)
            pb = p1.tile([DR, R], F32, tag="pb")
            nc.vector.tensor_tensor(pb[0:32], kr[0:32], sinl[0:32], OP.mult)
            nc.vector.tensor_tensor(pb[32:64], kr[32:64], cosl[32:64], OP.mult)
            sh_a = p1.tile([32, R], F32, tag="sha")
            nc.sync.dma_start(sh_a[:], pa[32:64])
            sh_b = p1.tile([32, R], F32, tag="shb")
            nc.sync.dma_start(sh_b[:], pb[32:64])
            out_r = p1.tile([32, R], F16, tag="outr")
            nc.vector.tensor_sub(out_r[:], pa[0:32], sh_a[:])
            out_i = p1.tile([32, R], F16, tag="outi")
            nc.vector.tensor_add(out_i[:], pb[0:32], sh_b[:])
            nc.sync.dma_start(ag_ckv_in[RKV:RKV + 32, :], out_r[:])
            nc.sync.dma_start(ag_ckv_in[RKV + 32:RKV + DR, :], out_i[:])
            # kv rmsnorm (local; ln folded into kv_b host-side)
            kinv = p1.tile([1, R], F32, tag="kinv")
            nc.vector.tensor_scalar(kinv[:], kss_ps[:], 1.0 / RKV, EPS,
                                    OP.mult, OP.add)
            nc.scalar.activation(kinv[:], kinv[:], AF.Sqrt)
            nc.vector.reciprocal(kinv[:], kinv[:])
            kinv_bc = p1.tile([P, R], F32, tag="kinvbc")
            nc.gpsimd.partition_broadcast(kinv_bc[:], kinv[:])
            for mb in range(RKB):
                outn = p1.tile([P, R], F16, tag="outn")
                nc.vector.tensor_tensor(outn[:], cp_all[:, mb, :], kinv_bc[:],
                                        OP.mult)
                nc.sync.dma_start(ag_ckv_in[mb * P:(mb + 1) * P, :], outn[:])
        ag("ag_ckv", ag_ckv_in, ag_ckv_out)

        # ---- q_a: RAW output in two pieces (4 + 8 blocks); squares
        # accumulate across both; 1/rms rides the tail of piece 1 ----
        def qa_piece(pc):
            with nc.named_scope(f"ph1_qa{pc}"):
                lo = QP[0] if pc else 0
                for i in range(QP[pc]):
                    mb = lo + i
                    ps = p1ps.tile([P, R], F32, tag="p1ps")
                    w = p1w.tile([P, HKB, P], F16, tag="w")
                    nc.sync.dma_start(w[:], qaw_t[mb])
                    for kb in range(HKB):
                        nc.tensor.matmul(ps[:], w[:, kb, :], hid_all(kb),
                                         start=(kb == 0), stop=(kb == HKB - 1))
                    raw = p1.tile([P, R], F16, tag="raw")
                    nc.vector.tensor_copy(raw[:], ps[:])
                    sq = p1sq.tile([P, R], F16, tag="sq")
                    nc.vector.tensor_tensor(sq[:], raw[:], raw[:], OP.mult)
                    nc.tensor.matmul(qss_ps[:], ones_col[:], sq[:],
                                     start=(mb == 0), stop=(mb == RQB - 1))
                    nc.sync.dma_start(ag_q_in[pc][i * P:(i + 1) * P, :], raw[:])

        for pc in range(2):
            qa_piece(pc)
            if pc == 1:
                qinv = p1.tile([1, R], F16, tag="qinv")
                qi32 = p1.tile([1, R], F32, tag="qi32")
                nc.vector.tensor_scalar(qi32[:], qss_ps[:], 1.0 / RQ, EPS,
                                        OP.mult, OP.add)
                nc.scalar.activation(qi32[:], qi32[:], AF.Sqrt)
                nc.vector.reciprocal(qi32[:], qi32[:])
                nc.vector.tensor_copy(qinv[:], qi32[:])
                nc.sync.dma_start(ag_q_in[1][QP[1] * P:QP[1] * P + 1, :],
                                  qinv[:])
            ag(f"ag_q{pc}", ag_q_in[pc], ag_q_out[pc])

        # later-phase weights: stream during the collective-wait window
        qbw = wres.tile([P, RQB, NHL * QK], F16, tag="qbw")
        for kb in range(RQB):
            nc.sync.dma_start(qbw[:, kb, :], qbw_t[kb])
        kvbw = wres.tile([P, RKB, NHL * (DN + DV)], F16, tag="kvbw")
        for b in range(RKB):
            nc.sync.dma_start(kvbw[:, b, :], kvbw_t[b])
        mask_sb = const.tile([P, 896], F16, tag="mask")
        nc.sync.dma_start(mask_sb[:], mask_in[:])
        p1ctx.close()
        hidp_ctx.close()

        # rope tables / mask (host-precomputed), prefetched during phase 1
        cos4 = const.tile([P, S], F32, tag="cos4")
        nc.sync.dma_start(cos4[:], cos4_in[:])
        sin4 = const.tile([P, S], F32, tag="sin4")
        nc.sync.dma_start(sin4[:], sin4_in[:])

        # ============ phase 2a then 2b (shared PSUM scope) ==================
        qres = ctx.enter_context(tc.tile_pool(name="qres", bufs=1))
        qpass = [qres.tile([P, S], F16, tag=f"qp{h}", name=f"qp{h}")
                 for h in range(NHL)]
        q_re = qres.tile([P, S], F16, tag="qre")   # [re_h0..re_h3] x32, roped
        q_im = qres.tile([P, S], F16, tag="qim")   # [im_h0..im_h3] x32, roped
        qinv_bc = qres.tile([P, S], F32, tag="qinvbc")

        kvres = ctx.enter_context(tc.tile_pool(name="kvres", bufs=1))
        kpass = [kvres.tile([P, S], F16, tag=f"kp{h}", name=f"kp{h}")
                 for h in range(NHL)]
        krot2 = kvres.tile([P, S], F16, tag="krot2")   # krot duplicated 2x64
        v_all = kvres.tile([P, KB, 512], F16, tag="v")

        # rope tables (host-precomputed)
        cos4 = const.tile([P, S], F32, tag="cos4")
        nc.sync.dma_start(cos4[:], cos4_in[:])
        sin4 = const.tile([P, S], F32, tag="sin4")
        nc.sync.dma_start(sin4[:], sin4_in[:])

        with (
            tc.tile_pool(name="ckvp", bufs=1) as ckvp,
            tc.tile_pool(name="qlp", bufs=1) as qlp,
            tc.tile_pool(name="qstg", bufs=2) as qstg,
            tc.tile_pool(name="ropep", bufs=2) as ropep,
            tc.tile_pool(name="p2ps", bufs=2, space="PSUM") as p2ps,
            tc.tile_pool(name="qbps", bufs=2, space="PSUM") as qbps,
        ):
            # gather-side loads, emitted in collective completion order
            ckv_sb = ckvp.tile([P, RKB, S], F16, tag="ckv")
            for b in range(RKB):
                nc.sync.dma_start(
                    ckv_sb[:, b, :].rearrange("p (r c) -> p r c", r=NCORES),
                    ag_ckv_out[:, b * P:(b + 1) * P, :].rearrange("r p c -> p r c"))
            for half in range(2):
                nc.sync.dma_start(
                    krot2[64 * half:64 * (half + 1), :]
                    .rearrange("p (r c) -> p r c", r=NCORES),
                    ag_ckv_out[:, RKV:RKV + DR, :].rearrange("r p c -> p r c"))
            ql_pc = [qlp.tile([P, QP[pc], S], F16, tag=f"ql{pc}",
                              name=f"ql{pc}") for pc in range(2)]
            for pc in range(2):
                for j in range(QP[pc]):
                    nc.sync.dma_start(
                        ql_pc[pc][:, j, :].rearrange("p (r c) -> p r c",
                                                     r=NCORES),
                        ag_q_out[pc][:, j * P:(j + 1) * P, :]
                        .rearrange("r p c -> p r c"))
            qiv = qlp.tile([1, S], F16, tag="qiv")
            nc.sync.dma_start(
                qiv[:].rearrange("p (r c) -> p r c", r=NCORES),
                ag_q_out[1][:, QP[1] * P:QP[1] * P + 1, :]
                .rearrange("r p c -> p r c"))
            qiv32 = qlp.tile([1, S], F32, tag="qiv32")
            nc.vector.tensor_copy(qiv32[:], qiv[:])
            nc.gpsimd.partition_broadcast(qinv_bc[:], qiv32[:])

            # ---- phase 2a: kv_b -> k_passT, v (runs during the q gathers) --
            with nc.named_scope("ph2_kvb"):
                for h in range(NHL):
                    for sb in range(SB):
                        ps = p2ps.tile([P, 512], F32, tag="ps2")
                        for b in range(RKB):
                            nc.tensor.matmul(
                                ps[:], kvbw[:, b, h * P:(h + 1) * P],
                                ckv_sb[:, b, sb * 512:(sb + 1) * 512],
                                start=(b == 0), stop=(b == RKB - 1))
                        nc.vector.tensor_copy(
                            kpass[h][:, sb * 512:(sb + 1) * 512], ps[:])
                for s in range(KB):
                    ps = p2ps.tile([P, 512], F32, tag="ps2")
                    for b in range(RKB):
                        nc.tensor.matmul(
                            ps[:], ckv_sb[:, b, s * P:(s + 1) * P],
                            kvbw[:, b, NHL * DN:],
                            start=(b == 0), stop=(b == RKB - 1))
                    nc.vector.tensor_copy(v_all[:, s, :], ps[:])

            # ---- phase 2b: q_b -> q_passT + roped q_rot --------------------
            with nc.named_scope("ph2_qb"):
                HQ = QMB // 2
                for sb in range(SB):
                    sl = slice(sb * 512, (sb + 1) * 512)
                    qre32 = qstg.tile([P, 512], F32, tag="qre32")
                    qim32 = qstg.tile([P, 512], F32, tag="qim32")
                    for rep in range(2):
                        pss = [qbps.tile([P, 512], F32, tag=f"qps{j}",
                                         name=f"qps{j}") for j in range(HQ)]
                        for kb in range(RQB):
                            ql = (ql_pc[0][:, kb, sl] if kb < QP[0]
                                  else ql_pc[1][:, kb - QP[0], sl])
                            for j in range(HQ):
                                m = rep * HQ + j
                                nc.tensor.matmul(
                                    pss[j][:], qbw[:, kb, m * P:(m + 1) * P],
                                    ql, start=(kb == 0), stop=(kb == RQB - 1))
                        for j in range(HQ):
                            m = rep * HQ + j
                            if m < NHL:
                                dst = qpass[m][:, sl]
                            elif m == NHL:
                                dst = qre32[:]
                            else:
                                dst = qim32[:]
                            nc.vector.tensor_tensor(dst, pss[j][:],
                                                    qinv_bc[:, sl], OP.mult)
                    t = ropep.tile([P, 512], F32, tag="t")
                    u = ropep.tile([P, 512], F32, tag="u")
                    t2 = ropep.tile([P, 512], F32, tag="t2")
                    nc.vector.tensor_tensor(t[:], qre32[:], cos4[:, sl], OP.mult)
                    nc.vector.tensor_tensor(u[:], qim32[:], sin4[:, sl], OP.mult)
                    nc.vector.tensor_tensor(t2[:], qre32[:], sin4[:, sl], OP.mult)
                    nc.vector.tensor_sub(q_re[:, sl], t[:], u[:])
                    nc.vector.tensor_tensor(u[:], qim32[:], cos4[:, sl], OP.mult)
                    nc.vector.tensor_add(q_im[:, sl], t2[:], u[:])

        # ============ phase 3+4: attention with interleaved o-proj ==========
        ores = ctx.enter_context(tc.tile_pool(name="ores", bufs=1))
        o_heads = [[ores.tile([P, 512], F16, tag=f"oh{h}_{qb}",
                              name=f"oh{h}_{qb}") for qb in range(SB)]
                   for h in range(NHL)]
        ow_sb = ores.tile([P, NOB, NHL, 512], F16, tag="owsb")
        for nb in range(NOB):
            nc.sync.dma_start(ow_sb[:, nb, :, :], ow_t[nb])

        jobs = deque()

        with (
            tc.tile_pool(name="scps", bufs=2, space="PSUM") as scps,
            tc.tile_pool(name="ops", bufs=1, space="PSUM") as ops,
            tc.tile_pool(name="smps", bufs=1, space="PSUM") as smps,
            tc.tile_pool(name="ojps", bufs=1, space="PSUM") as ojps,
            tc.tile_pool(name="att", bufs=3) as att,
            tc.tile_pool(name="atts", bufs=4) as atts,
            tc.tile_pool(name="attq", bufs=2) as attq,
            tc.tile_pool(name="oj", bufs=3) as oj,
            nc.named_scope("ph34"),
        ):
            def emit_job():
                qb, s, nb = jobs.popleft()
                ps = ojps.tile([P, 512], F32, tag="ojps")
                for h in range(NHL):
                    nc.tensor.matmul(
                        ps[:], o_heads[h][qb][:, s * P:(s + 1) * P],
                        ow_sb[:, nb, h, :],
                        start=(h == 0), stop=(h == NHL - 1))
                ot = oj.tile([P, 512], F16, tag="ot")
                nc.vector.tensor_copy(ot[:], ps[:])
                nc.sync.dma_start(
                    o_part[qb * 512 + s * P: qb * 512 + (s + 1) * P,
                           nb * 512:(nb + 1) * 512], ot[:])

            def drain(k):
                for _ in range(min(k, len(jobs))):
                    emit_job()

            for qb in range(SB):
                nkb = (qb + 1) * 4
                for hp in range(NHL // 2):
                    h0, h1 = 2 * hp, 2 * hp + 1
                    # packed roped-q for the pair: [re0;im0;re1;im1] x32
                    qrs = attq.tile([P, 512], F16, tag="qrs")
                    for j, h in ((0, h0), (2, h1)):
                        nc.sync.dma_start(
                            qrs[32 * j:32 * (j + 1), :],
                            q_re[32 * h:32 * (h + 1), qb * 512:(qb + 1) * 512])
                        nc.sync.dma_start(
                            qrs[32 * (j + 1):32 * (j + 2), :],
                            q_im[32 * h:32 * (h + 1), qb * 512:(qb + 1) * 512])
                    o_ps = [ops.tile([P, 512], F32, tag=f"ops{j}",
                                     name=f"ops{j}") for j in range(2)]
                    spart = [atts.tile([P, 512], F16, tag=f"sp{j}",
                                       name=f"sp{j}") for j in range(2)]
                    for kb in range(nkb):
                        k_sl = slice(kb * P, (kb + 1) * P)
                        d = kb * P - qb * 512
                        w0 = max(d, 0)
                        psl = slice(w0, 512)
                        qsl = slice(qb * 512 + w0, (qb + 1) * 512)
                        s_ps = scps.tile([P, 2, 512], F32, tag="sps")
                        for j, h in ((0, h0), (1, h1)):
                            nc.tensor.matmul(s_ps[:, j, psl],
                                             kpass[h][:, k_sl],
                                             qpass[h][:, qsl],
                                             start=True, stop=False)
                        for j in range(2):
                            nc.tensor.matmul(s_ps[:, j, psl],
                                             krot2[64 * j:64 * (j + 1), k_sl],
                                             qrs[64 * j:64 * (j + 1), psl],
                                             start=False, stop=True)
                        probs = att.tile([P, 2, 512], F16, tag="probs")
                        nc.scalar.activation(probs[:, :, psl], s_ps[:, :, psl],
                                             AF.Exp, scale=SCALE)
                        for j in range(2):
                            if d >= 0:
                                nc.vector.tensor_tensor(
                                    probs[:, j, w0:w0 + P],
                                    probs[:, j, w0:w0 + P],
                                    mask_sb[:, 384:512], OP.mult)
                            if kb == 0:
                                nc.vector.tensor_copy(spart[j][:],
                                                      probs[:, j, :])
                            else:
                                nc.vector.tensor_tensor(spart[j][:, psl],
                                                        spart[j][:, psl],
                                                        probs[:, j, psl],
                                                        OP.add)
                            nc.tensor.matmul(o_ps[j][:, psl],
                                             v_all[:, kb, (h0 + j) * P:
                                                   (h0 + j + 1) * P],
                                             probs[:, j, psl],
                                             start=(kb == 0),
                                             stop=(kb == nkb - 1))
                        drain(1)
                    for j in range(2):
                        sm_ps = smps.tile([1, 512], F32, tag="smps")
                        nc.tensor.matmul(sm_ps[:], ones_col[:], spart[j][:],
                                         start=True, stop=True)
                        rec = att.tile([1, 512], F32, tag="rec")
                        nc.vector.reciprocal_approx_fast(rec[:], sm_ps[:])
                        rec_bc = att.tile([P, 512], F32, tag="recbc")
                        nc.gpsimd.partition_broadcast(rec_bc[:], rec[:])
                        nc.vector.tensor_tensor(o_heads[h0 + j][qb][:],
                                                o_ps[j][:], rec_bc[:],
                                                OP.mult)
                    drain(2)
                for s in range(4):
                    for nb in range(NOB):
                        jobs.append((qb, s, nb))
            drain(len(jobs))


# ======================= host-side prep & entry ==========================

def prep_inputs(hidden_states, freqs, q_a_w, q_a_ln_w, q_b_w, kv_a_w,
                kv_a_ln_w, kv_b_w, o_w):
    S = hidden_states.shape[1]
    R = S // NCORES
    f32, f16 = np.float32, np.float16

    hidT = np.ascontiguousarray(hidden_states[0].T.astype(f32))      # [H, S]
    freqsT = np.ascontiguousarray(freqs[0].T.astype(f32))            # [32, S]
    cos4 = np.ascontiguousarray(np.tile(np.cos(freqsT), (4, 1)))
    sin4 = np.ascontiguousarray(np.tile(np.sin(freqsT), (4, 1)))

    qawT = q_a_w.astype(f32).T                                       # [H, RQ]
    qaw_t = np.ascontiguousarray(
        qawT.reshape(H // P, P, RQ // P, P).transpose(2, 1, 0, 3).astype(f16))

    kva = kv_a_w.astype(f32)
    kva_main = kva[:RKV]
    kva_rot = np.concatenate([kva[RKV:][0::2], kva[RKV:][1::2]], axis=0)
    kvaw_t = np.ascontiguousarray(
        kva_main.T.reshape(H // P, P, RKV // P, P).transpose(2, 1, 0, 3).astype(f16))
    kvaw_rot_t = np.ascontiguousarray(
        kva_rot.T.reshape(H // P, P, DR).transpose(1, 0, 2).astype(f16))

    # big causal mask [128, 896]: M[p, g] = 1 if g >= p + 384
    g = np.arange(896)[None, :]
    p = np.arange(P)[:, None]
    mask = (g >= p + 384).astype(f16)

    # fold layernorm weights into the B matrices (exact: rmsnorm's 1/rms is
    # applied separately; the elementwise ln scale commutes into B columns)
    qbw = q_b_w.astype(f32) * q_a_ln_w.astype(f32)[None, :]
    kvbw = kv_b_w.astype(f32) * kv_a_ln_w.astype(f32)[None, :]
    ow = o_w.astype(f32)

    in_maps = []
    for c in range(NCORES):
        heads = list(range(NHL * c, NHL * (c + 1)))
        qb_pass = np.concatenate([qbw[QK * h:QK * h + DN] for h in heads], axis=0)
        qb_re = np.concatenate(
            [qbw[QK * h + DN:QK * (h + 1)][0::2] for h in heads], axis=0)
        qb_im = np.concatenate(
            [qbw[QK * h + DN:QK * (h + 1)][1::2] for h in heads], axis=0)
        qb_core = np.concatenate([qb_pass, qb_re, qb_im], axis=0)    # [768, RQ]
        qbw_tc = np.ascontiguousarray(
            qb_core.T.reshape(RQ // P, P, NHL * QK).astype(f16))
        kp = np.concatenate(
            [kvbw[(DN + DV) * h:(DN + DV) * h + DN] for h in heads], axis=0)
        vv = np.concatenate(
            [kvbw[(DN + DV) * h + DN:(DN + DV) * (h + 1)] for h in heads], axis=0)
        kvb_core = np.concatenate([kp, vv], axis=0)                  # [1024, RKV]
        kvbw_tc = np.ascontiguousarray(
            kvb_core.T.reshape(RKV // P, P, NHL * (DN + DV)).astype(f16))
        ow_slice = ow[:, NHL * DV * c: NHL * DV * (c + 1)].T         # [512, H]
        ow_tc = np.ascontiguousarray(
            ow_slice.reshape(NHL, P, H // 512, 512).transpose(2, 1, 0, 3).astype(f16))

        hid_c = np.ascontiguousarray(
            hidT[:, R * c:R * (c + 1)].reshape(H // P, P, R)
            .transpose(1, 0, 2).astype(f16))
        fl = freqsT[:, R * c:R * (c + 1)]
        cosl_c = np.ascontiguousarray(np.tile(np.cos(fl), (2, 1)))
        sinl_c = np.ascontiguousarray(np.tile(np.sin(fl), (2, 1)))

        in_maps.append({
            "hid_t": hid_c,
            "cos4": cos4,
            "sin4": sin4,
            "cosl": cosl_c,
            "sinl": sinl_c,
            "qaw_t": qaw_t,
            "kvaw_t": kvaw_t,
            "kvaw_rot": kvaw_rot_t,
            "qbw_t": qbw_tc,
            "kvbw_t": kvbw_tc,
            "ow_t": ow_tc,
            "mask": mask,
        })
    return in_maps


def _run(inputs, trace=False, trace_kwargs=None):
    S = inputs["hidden_states"].shape[1]
    if S not in _BUILD_CACHE:
        _BUILD_CACHE[S] = build(S)
    nc = _BUILD_CACHE[S]
    in_maps = prep_inputs(**inputs)
    kw = {}
    if trace:
        kw["trace"] = True
        if trace_kwargs:
            kw.update(trace_kwargs)
    res = run_bass_kernel_spmd(nc, in_maps, list(range(NCORES)), **kw)
    parts = np.stack([r["o_part"] for r in res.results], axis=0)
    out = parts.astype(np.float64).sum(axis=0).astype(np.float32)
    return out[None], res


def kernel(**inputs):
    out, _ = _run(inputs)
    return out
